# revision 23
# baseline (speedup 1.0000x reference)
"""Single-head attention (B=8, T=2048, E=1024, D=128) on 8 Trainium2 NeuronCores.

Strategy (data-parallel over batch, one batch element per core):
  host: pre-transpose x -> xT[b] = x[b].T (E on rows) so the device needs no
        large transposes; pre-scale q/k biases by D**-0.25.
  device, per core:
    - load Wq/Wk/Wv, biases, xT chunks (E-partitioned) into SBUF
    - qT/kT/vT = W.T @ xT via PE (f32r matmuls, N=512 -> full rate),
      bias folded into the PSUM->SBUF copy on ACT; q,k scaled by D**-0.25
    - V (natural [k, D] layout) from vT via 16 PE transposes
    - per 512-wide query span:
        ST[k_blk, q] = kT_blk.T @ qT_span   (scores, transposed, PSUM)
        P = exp(ST) on ACT (PSUM->SBUF); unnormalized softmax (no max
        subtraction needed: scores are O(5) by construction)
        OT[d, q] += V_blk.T @ P_blk         (attention output, transposed)
        l[q] = column sums of P via DVE partition-fold; broadcast with a
        rank-1 PE matmul; out_span = OT * (1/l) on DVE
    - store outT [D, T]; host transposes back to [T, D].
"""

import os
import sys

for _p in ("/opt/trn_rl_repo",):
    if _p not in sys.path and os.path.isdir(_p):
        sys.path.append(_p)

import numpy as np

import concourse.bass as bass
import concourse.tile as tile
from concourse import mybir
from concourse.masks import make_identity
from concourse.vector_clock import ScopedClock

B, T, E, D = 8, 2048, 1024, 128
EC = E // 128          # E chunks of 128 partitions
NSPAN = 4              # query spans of 512
SPAN = T // NSPAN      # 512
NKB = T // 128         # 16 key blocks
F32 = mybir.dt.float32
F32R = mybir.dt.float32r
BF16 = mybir.dt.bfloat16

_MAX_DRAIN_WAITS = 1


def _drain_and_barrier_split(self, tick_clock, wait_clock):
    # This walrus build rejects CTRL instructions carrying more than one sync
    # wait, so spread the kernel-tail drain's waits over single-wait NOPs.
    nc = self.nc
    collector = nc.sync.nop(nofuse=True, hint="drain_wait_collector")
    wait_clock.add_sem_waits(
        collector.ins, ScopedClock({None: tick_clock.global_clock})
    )
    si = collector.ins.sync_info
    waits = list(si.on_wait) if si and si.on_wait else []
    if len(waits) > _MAX_DRAIN_WAITS:
        si.on_wait = waits[:_MAX_DRAIN_WAITS]
        rest = waits[_MAX_DRAIN_WAITS:]
        while rest:
            chunk, rest = rest[:_MAX_DRAIN_WAITS], rest[_MAX_DRAIN_WAITS:]
            extra = nc.sync.nop(nofuse=True, hint="drain_wait_extra")
            if extra.ins.sync_info is None:
                extra.ins.sync_info = type(si)(on_wait=chunk, on_update=[])
            else:
                extra.ins.sync_info.on_wait = chunk

    nc.sync.drain()

    nc.all_engine_barrier()
    assert self.sems is not None
    popped = nc._tile_sem_poison_stack.pop()
    assert popped is self._sem_poison
    nc.clear_and_free_semaphores(list(self.sems.allocated().values()))
    nc.all_engine_barrier()


tile.TileContext._drain_and_barrier = _drain_and_barrier_split


def _split_excess_waits(nc):
    """Walrus in this env allows at most one sync wait per instruction;
    hoist extra waits onto same-engine NOPs placed just before."""
    import copy

    m = nc.m
    cnt = 0
    new_funcs = []
    for function in m.functions:
        new_function = copy.replace(function, blocks=[])
        new_function.set_allocations_from_list(function.allocations)
        for block in function.blocks:
            new_insts = []
            for inst in block.instructions:
                si = inst.sync_info
                waits = list(si.on_wait) if si and si.on_wait else []
                if len(waits) > 1:
                    for w in waits[:-1]:
                        nop = mybir.InstNoOp(name=f"I-swsplit-{cnt}",
                                             ins=[], outs=[])
                        cnt += 1
                        nop.engine = inst.engine
                        nop.sync_info = mybir.SyncInfo(on_wait=[w],
                                                       on_update=[])
                        new_insts.append(nop)
                    si.on_wait = [waits[-1]]
                new_insts.append(inst)
            new_function.blocks.append(
                copy.replace(block, instructions=new_insts))
        new_funcs.append(new_function)
    new_m = copy.replace(m, functions=[])
    for f in new_funcs:
        new_m.functions.append(f)
    nc.m = new_m
    return cnt


def build_nc(mm_dt=F32R, variant=None):
    variant = variant or os.environ.get("KVARIANT", "full")
    SCALE = float(np.float32(D) ** np.float32(-0.25))
    if mm_dt == "mixed":
        DTM = F32R         # scores path (x, W, qT, kT): tf32
        PVDT = BF16        # P and V for the attention-output matmul
    else:
        DTM = mm_dt        # dtype for matmul operands (f32r = tf32 on PE)
        PVDT = mm_dt

    def mm(ap):
        return ap

    def f32view(ap):
        # DVE reads bf16 operands natively; f32r needs a bit-identical view
        return ap.bitcast(F32) if ap.dtype == F32R else ap

    nc = bass.Bass()
    xT = nc.declare_dram_parameter("xT", [E, T], DTM, isOutput=False)[:]
    Wq = nc.declare_dram_parameter("Wq", [128, EC * D], DTM, isOutput=False)[:]
    Wk = nc.declare_dram_parameter("Wk", [128, EC * D], DTM, isOutput=False)[:]
    Wv = nc.declare_dram_parameter("Wv", [128, EC * D], DTM, isOutput=False)[:]
    bqc = nc.declare_dram_parameter("bqc", [D], F32, isOutput=False)[:]
    bkc = nc.declare_dram_parameter("bkc", [D], F32, isOutput=False)[:]
    bv = nc.declare_dram_parameter("bv", [D], F32, isOutput=False)[:]
    outT = nc.declare_dram_parameter("outT", [D, T], F32, isOutput=True)[:]

    with tile.TileContext(nc) as tc, \
         tc.tile_pool(name="consts", bufs=1) as consts, \
         tc.tile_pool(name="xpool", bufs=1) as xpool, \
         tc.tile_pool(name="persist", bufs=1) as persist, \
         tc.tile_pool(name="stpool", bufs=6) as stpool, \
         tc.tile_pool(name="accpool", bufs=2) as accpool, \
         tc.tile_pool(name="lbpool", bufs=2) as lbpool, \
         tc.tile_pool(name="outpool", bufs=2) as outpool, \
         tc.tile_pool(name="psA", bufs=2, space="PSUM") as psA, \
         tc.tile_pool(name="psB", bufs=4, space="PSUM") as psB:

        # ---- constants / weights ----
        wq_s = consts.tile([128, EC, D], DTM, tag="wq")
        wk_s = consts.tile([128, EC, D], DTM, tag="wk")
        wv_s = consts.tile([128, EC, D], DTM, tag="wv")
        for w_s, w_d in ((wq_s, Wq), (wk_s, Wk), (wv_s, Wv)):
            nc.sync.dma_start(
                out=w_s, in_=w_d.rearrange("p (c d) -> p c d", d=D)
            )
        bq_s = consts.tile([128, 1], F32, tag="bq")
        bk_s = consts.tile([128, 1], F32, tag="bk")
        bv_s = consts.tile([128, 1], F32, tag="bv")
        for b_s, b_d in ((bq_s, bqc), (bk_s, bkc), (bv_s, bv)):
            nc.sync.dma_start(out=b_s, in_=b_d.unsqueeze(1))
        TDT = F32
        ident = consts.tile([128, 128], TDT, tag="ident")
        make_identity(nc, ident)
        ones_mat = consts.tile([128, 128], F32, tag="ones_mat")
        nc.gpsimd.memset(ones_mat, 1.0)

        # ---- x chunks (E-partitioned), streamed ----
        xs = []
        for e in range(EC):
            xe = xpool.tile([128, T], DTM, tag=f"xt{e}")
            nc.sync.dma_start(out=xe, in_=xT[e * 128:(e + 1) * 128, :])
            xs.append(xe)

        kT_s = persist.tile([128, T], DTM, tag="kT")
        vT_s = persist.tile([128, T], TDT, tag="vT")
        qT_s = persist.tile([128, T], DTM, tag="qT")
        V_s = persist.tile([128, NKB, D], PVDT, tag="V")

        # ---- k/v projections, h-halves of T so PE consumes chunks as they
        # arrive (accumulate over E in PSUM) ----
        for h in range(2):
            k_ps = psA.tile([128, T // 2], F32, tag="mm")
            v_ps = psA.tile([128, T // 2], F32, tag="mm")
            for e in range(EC):
                for w_s, ps in ((wk_s, k_ps), (wv_s, v_ps)):
                    for j in range(2):  # two 512-wide slices of this half
                        sl = slice(j * 512, (j + 1) * 512)
                        xsl = xs[e][:, h * (T // 2) + j * 512:
                                    h * (T // 2) + (j + 1) * 512]
                        nc.tensor.matmul(ps[:, sl], mm(w_s[:, e, :]),
                                         mm(xsl), start=(e == 0),
                                         stop=(e == EC - 1))
            hsl = slice(h * (T // 2), (h + 1) * (T // 2))
            nc.vector.tensor_scalar(out=kT_s[:, hsl], in0=k_ps,
                                    scalar1=bk_s, scalar2=SCALE,
                                    op0=mybir.AluOpType.add,
                                    op1=mybir.AluOpType.mult)
            nc.vector.tensor_scalar(out=vT_s[:, hsl], in0=v_ps,
                                    scalar1=bv_s, scalar2=None,
                                    op0=mybir.AluOpType.add)

        # ---- V natural layout [k, D] via PE transposes of vT ----
        for t in range(NKB):
            vt_ps = psB.tile([128, 128], TDT, tag="small")
            nc.tensor.transpose(vt_ps, vT_s[:, t * 128:(t + 1) * 128], ident)
            nc.vector.tensor_copy(out=V_s[:, t, :], in_=vt_ps)

        # ---- per query span: q proj, scores^T, exp, O^T, normalize ----
        for s in range(NSPAN):
            ssl = slice(s * SPAN, (s + 1) * SPAN)
            q_ps = psB.tile([128, SPAN], F32, tag="small")
            for e in range(EC):
                nc.tensor.matmul(q_ps, mm(wq_s[:, e, :]), mm(xs[e][:, ssl]),
                                 start=(e == 0), stop=(e == EC - 1))
            nc.vector.tensor_scalar(out=qT_s[:, ssl], in0=q_ps,
                                    scalar1=bq_s, scalar2=SCALE,
                                    op0=mybir.AluOpType.add,
                                    op1=mybir.AluOpType.mult)

            ot_ps = psB.tile([128, SPAN], F32, tag="small")
            acc = accpool.tile([128, SPAN], F32, tag="acc")
            prev_st = None
            for kb in range(NKB):
                st_ps = psA.tile([128, SPAN], F32, tag="mm")
                nc.tensor.matmul(st_ps,
                                 mm(kT_s[:, kb * 128:(kb + 1) * 128]),
                                 mm(qT_s[:, ssl]), start=True, stop=True)
                st_x = stpool.tile([128, SPAN], PVDT, tag="st")
                nc.scalar.activation(out=st_x, in_=st_ps,
                                     func=mybir.ActivationFunctionType.Exp)
                if kb == 0:
                    nc.vector.tensor_copy(out=acc, in_=f32view(st_x))
                else:
                    nc.vector.tensor_add(out=acc, in0=acc, in1=f32view(st_x))
                if prev_st is not None:
                    kbp, stp = prev_st
                    nc.tensor.matmul(ot_ps, mm(V_s[:, kbp, :]), mm(stp),
                                     start=(kbp == 0), stop=False)
                prev_st = (kb, st_x)
            kbp, stp = prev_st
            nc.tensor.matmul(ot_ps, mm(V_s[:, kbp, :]), mm(stp),
                             start=False, stop=True)

            outsp = outpool.tile([128, SPAN], F32, tag="out")
            if variant == "nonorm":
                nc.vector.tensor_copy(out=outsp, in_=ot_ps)
            else:
                # lb[d, q] = sum over partitions of acc, broadcast to all
                # 128 partitions, via one all-ones 128x128 matmul (exact f32)
                lb_ps = psB.tile([128, SPAN], F32, tag="small")
                nc.tensor.matmul(lb_ps, ones_mat, acc,
                                 start=True, stop=True)
                lb = lbpool.tile([128, SPAN], F32, tag="lb")
                nc.vector.reciprocal(out=lb, in_=lb_ps)
                nc.vector.tensor_mul(out=outsp, in0=ot_ps, in1=lb)
            nc.sync.dma_start(out=outT[:, ssl], in_=outsp)

    return nc


_CACHED = {}


def _get_nc(mm_dt=F32R):
    key = str(mm_dt)
    if key not in _CACHED:
        nc = build_nc(mm_dt)
        _split_excess_waits(nc)
        _CACHED[key] = nc
    return _CACHED[key]


def _round_tf32(a):
    """Round fp32 to tf32 (10-bit mantissa), round-to-nearest-even."""
    u = np.ascontiguousarray(a, np.float32).view(np.uint32)
    r = (u + np.uint32(0xFFF) + ((u >> np.uint32(13)) & np.uint32(1))) \
        & np.uint32(0xFFFFE000)
    return r.view(np.float32)


def _make_in_maps(x, Wq, bq, Wk, bk, Wv, bv, mm_dt=F32R):
    rnd = _round_tf32 if mm_dt == F32R else (
        lambda a: np.ascontiguousarray(a, np.float32))
    xT = rnd(np.transpose(np.asarray(x, np.float32), (0, 2, 1)))

    def warr(w):
        w = np.asarray(w, np.float32).reshape(EC, 128, D)
        return rnd(w.transpose(1, 0, 2).reshape(128, EC * D))

    Wq, Wk, Wv = warr(Wq), warr(Wk), warr(Wv)
    bqc = np.ascontiguousarray(np.asarray(bq, np.float32))
    bkc = np.ascontiguousarray(np.asarray(bk, np.float32))
    bv = np.ascontiguousarray(np.asarray(bv, np.float32))
    return [
        {"xT": np.ascontiguousarray(xT[b]), "Wq": Wq, "Wk": Wk, "Wv": Wv,
         "bqc": bqc, "bkc": bkc, "bv": bv}
        for b in range(B)
    ]


def kernel(x, Wq, bq, Wk, bk, Wv, bv, _trace=False, _mm_dt=None):
    from concourse.bass_utils import run_bass_kernel_spmd

    mm_dt = _mm_dt if _mm_dt is not None else F32R
    nc = _get_nc(mm_dt)
    in_maps = _make_in_maps(x, Wq, bq, Wk, bk, Wv, bv, mm_dt=mm_dt)
    res = run_bass_kernel_spmd(nc, in_maps, core_ids=list(range(B)),
                               trace=_trace)
    out = np.stack([np.ascontiguousarray(res.results[b]["outT"].T)
                    for b in range(B)])
    kernel._last_result = res
    return out


# revision 25
# speedup vs baseline: 1.1294x; 1.1294x over previous
"""Single-head attention (B=8, T=2048, E=1024, D=128) on 8 Trainium2 NeuronCores.

Strategy (data-parallel over batch, one batch element per core):
  host: pre-transpose x -> xT[b] = x[b].T (E on rows) so the device needs no
        large transposes; pre-scale q/k biases by D**-0.25.
  device, per core:
    - load Wq/Wk/Wv, biases, xT chunks (E-partitioned) into SBUF
    - qT/kT/vT = W.T @ xT via PE (f32r matmuls, N=512 -> full rate),
      bias folded into the PSUM->SBUF copy on ACT; q,k scaled by D**-0.25
    - V (natural [k, D] layout) from vT via 16 PE transposes
    - per 512-wide query span:
        ST[k_blk, q] = kT_blk.T @ qT_span   (scores, transposed, PSUM)
        P = exp(ST) on ACT (PSUM->SBUF); unnormalized softmax (no max
        subtraction needed: scores are O(5) by construction)
        OT[d, q] += V_blk.T @ P_blk         (attention output, transposed)
        l[q] = column sums of P via DVE partition-fold; broadcast with a
        rank-1 PE matmul; out_span = OT * (1/l) on DVE
    - store outT [D, T]; host transposes back to [T, D].
"""

import os
import sys

for _p in ("/opt/trn_rl_repo",):
    if _p not in sys.path and os.path.isdir(_p):
        sys.path.append(_p)

import numpy as np

import concourse.bass as bass
import concourse.tile as tile
from concourse import mybir
from concourse.masks import make_identity
from concourse.vector_clock import ScopedClock

B, T, E, D = 8, 2048, 1024, 128
EC = E // 128          # E chunks of 128 partitions
NSPAN = 4              # query spans of 512
SPAN = T // NSPAN      # 512
NKB = T // 128         # 16 key blocks
F32 = mybir.dt.float32
F32R = mybir.dt.float32r
BF16 = mybir.dt.bfloat16

_MAX_DRAIN_WAITS = 1


def _drain_and_barrier_split(self, tick_clock, wait_clock):
    # This walrus build rejects CTRL instructions carrying more than one sync
    # wait, so spread the kernel-tail drain's waits over single-wait NOPs.
    nc = self.nc
    collector = nc.sync.nop(nofuse=True, hint="drain_wait_collector")
    wait_clock.add_sem_waits(
        collector.ins, ScopedClock({None: tick_clock.global_clock})
    )
    si = collector.ins.sync_info
    waits = list(si.on_wait) if si and si.on_wait else []
    if len(waits) > _MAX_DRAIN_WAITS:
        si.on_wait = waits[:_MAX_DRAIN_WAITS]
        rest = waits[_MAX_DRAIN_WAITS:]
        while rest:
            chunk, rest = rest[:_MAX_DRAIN_WAITS], rest[_MAX_DRAIN_WAITS:]
            extra = nc.sync.nop(nofuse=True, hint="drain_wait_extra")
            if extra.ins.sync_info is None:
                extra.ins.sync_info = type(si)(on_wait=chunk, on_update=[])
            else:
                extra.ins.sync_info.on_wait = chunk

    nc.sync.drain()

    nc.all_engine_barrier()
    assert self.sems is not None
    popped = nc._tile_sem_poison_stack.pop()
    assert popped is self._sem_poison
    nc.clear_and_free_semaphores(list(self.sems.allocated().values()))
    nc.all_engine_barrier()


tile.TileContext._drain_and_barrier = _drain_and_barrier_split


def _split_excess_waits(nc):
    """Walrus in this env allows at most one sync wait per instruction;
    hoist extra waits onto same-engine NOPs placed just before."""
    import copy

    m = nc.m
    cnt = 0
    new_funcs = []
    for function in m.functions:
        new_function = copy.replace(function, blocks=[])
        new_function.set_allocations_from_list(function.allocations)
        for block in function.blocks:
            new_insts = []
            for inst in block.instructions:
                si = inst.sync_info
                waits = list(si.on_wait) if si and si.on_wait else []
                if len(waits) > 1:
                    for w in waits[:-1]:
                        nop = mybir.InstNoOp(name=f"I-swsplit-{cnt}",
                                             ins=[], outs=[])
                        cnt += 1
                        nop.engine = inst.engine
                        nop.sync_info = mybir.SyncInfo(on_wait=[w],
                                                       on_update=[])
                        new_insts.append(nop)
                    si.on_wait = [waits[-1]]
                new_insts.append(inst)
            new_function.blocks.append(
                copy.replace(block, instructions=new_insts))
        new_funcs.append(new_function)
    new_m = copy.replace(m, functions=[])
    for f in new_funcs:
        new_m.functions.append(f)
    nc.m = new_m
    return cnt


def build_nc(mm_dt=F32R, variant=None):
    variant = variant or os.environ.get("KVARIANT", "full")
    SCALE = float(np.float32(D) ** np.float32(-0.25))
    if mm_dt == "mixed":
        DTM = F32R         # x, W (projection operands)
        QKDT = F32R        # qT, kT (scores operands): tf32
        PVDT = BF16        # P and V for the attention-output matmul
    elif mm_dt == "bq":
        DTM = BF16
        QKDT = F32R
        PVDT = BF16
    else:
        DTM = mm_dt        # dtype for matmul operands (f32r = tf32 on PE)
        QKDT = mm_dt
        PVDT = mm_dt

    def mm(ap):
        return ap

    def f32view(ap):
        # DVE reads bf16 operands natively; f32r needs a bit-identical view
        return ap.bitcast(F32) if ap.dtype == F32R else ap

    nc = bass.Bass()
    xT = nc.declare_dram_parameter("xT", [E, T], DTM, isOutput=False)[:]
    Wq = nc.declare_dram_parameter("Wq", [128, EC * D], DTM, isOutput=False)[:]
    Wk = nc.declare_dram_parameter("Wk", [128, EC * D], DTM, isOutput=False)[:]
    Wv = nc.declare_dram_parameter("Wv", [128, EC * D], DTM, isOutput=False)[:]
    bqc = nc.declare_dram_parameter("bqc", [D], F32, isOutput=False)[:]
    bkc = nc.declare_dram_parameter("bkc", [D], F32, isOutput=False)[:]
    bv = nc.declare_dram_parameter("bv", [D], F32, isOutput=False)[:]
    outT = nc.declare_dram_parameter("outT", [D, T], F32, isOutput=True)[:]

    with tile.TileContext(nc) as tc, \
         tc.tile_pool(name="consts", bufs=1) as consts, \
         tc.tile_pool(name="xpool", bufs=1) as xpool, \
         tc.tile_pool(name="persist", bufs=1) as persist, \
         tc.tile_pool(name="stpool", bufs=6) as stpool, \
         tc.tile_pool(name="accpool", bufs=2) as accpool, \
         tc.tile_pool(name="lbpool", bufs=2) as lbpool, \
         tc.tile_pool(name="outpool", bufs=2) as outpool, \
         tc.tile_pool(name="psA", bufs=3, space="PSUM") as psA, \
         tc.tile_pool(name="psB", bufs=2, space="PSUM") as psB:

        # ---- constants / weights ----
        wq_s = consts.tile([128, EC, D], DTM, tag="wq")
        wk_s = consts.tile([128, EC, D], DTM, tag="wk")
        wv_s = consts.tile([128, EC, D], DTM, tag="wv")
        for w_s, w_d in ((wq_s, Wq), (wk_s, Wk), (wv_s, Wv)):
            nc.sync.dma_start(
                out=w_s, in_=w_d.rearrange("p (c d) -> p c d", d=D)
            )
        bq_s = consts.tile([128, 1], F32, tag="bq")
        bk_s = consts.tile([128, 1], F32, tag="bk")
        bv_s = consts.tile([128, 1], F32, tag="bv")
        for b_s, b_d in ((bq_s, bqc), (bk_s, bkc), (bv_s, bv)):
            nc.sync.dma_start(out=b_s, in_=b_d.unsqueeze(1))
        TDT = F32
        ident = consts.tile([128, 128], TDT, tag="ident")
        make_identity(nc, ident)
        ones_mat = consts.tile([128, 128], F32, tag="ones_mat")
        nc.gpsimd.memset(ones_mat, 1.0)

        # ---- x chunks (E-partitioned), streamed ----
        xs = []
        for e in range(EC):
            xe = xpool.tile([128, T], DTM, tag=f"xt{e}")
            nc.sync.dma_start(out=xe, in_=xT[e * 128:(e + 1) * 128, :])
            xs.append(xe)

        kT_s = persist.tile([128, T], QKDT, tag="kT")
        vT_s = persist.tile([128, T], TDT, tag="vT")
        qT_s = persist.tile([128, T], QKDT, tag="qT")
        V_s = persist.tile([128, NKB, D], PVDT, tag="V")

        # ---- k/v projections, h-halves of T so PE consumes chunks as they
        # arrive (accumulate over E in PSUM) ----
        for h in range(2):
            k_ps = psA.tile([128, T // 2], F32, tag="mm")
            v_ps = psA.tile([128, T // 2], F32, tag="mm")
            q_ps = psA.tile([128, T // 2], F32, tag="mm")
            for e in range(EC):
                for w_s, ps in ((wk_s, k_ps), (wv_s, v_ps), (wq_s, q_ps)):
                    for j in range(2):  # two 512-wide slices of this half
                        sl = slice(j * 512, (j + 1) * 512)
                        xsl = xs[e][:, h * (T // 2) + j * 512:
                                    h * (T // 2) + (j + 1) * 512]
                        nc.tensor.matmul(ps[:, sl], mm(w_s[:, e, :]),
                                         mm(xsl), start=(e == 0),
                                         stop=(e == EC - 1))
            hsl = slice(h * (T // 2), (h + 1) * (T // 2))
            nc.vector.tensor_scalar(out=kT_s[:, hsl], in0=k_ps,
                                    scalar1=bk_s, scalar2=SCALE,
                                    op0=mybir.AluOpType.add,
                                    op1=mybir.AluOpType.mult)
            nc.vector.tensor_scalar(out=vT_s[:, hsl], in0=v_ps,
                                    scalar1=bv_s, scalar2=None,
                                    op0=mybir.AluOpType.add)
            nc.vector.tensor_scalar(out=qT_s[:, hsl], in0=q_ps,
                                    scalar1=bq_s, scalar2=SCALE,
                                    op0=mybir.AluOpType.add,
                                    op1=mybir.AluOpType.mult)

        # ---- V natural layout [k, D] via PE transposes of vT ----
        for t in range(NKB):
            vt_ps = psB.tile([128, 128], TDT, tag="small")
            nc.tensor.transpose(vt_ps, vT_s[:, t * 128:(t + 1) * 128], ident)
            nc.vector.tensor_copy(out=V_s[:, t, :], in_=vt_ps)

        # ---- per query span: q proj, scores^T, exp, O^T, normalize ----
        for s in range(NSPAN):
            ssl = slice(s * SPAN, (s + 1) * SPAN)
            ot_ps = psB.tile([128, SPAN], F32, tag="small")
            acc = accpool.tile([128, SPAN], F32, tag="acc")
            prev_st = None
            for kb in range(NKB):
                st_ps = psA.tile([128, SPAN], F32, tag="mm")
                nc.tensor.matmul(st_ps,
                                 mm(kT_s[:, kb * 128:(kb + 1) * 128]),
                                 mm(qT_s[:, ssl]), start=True, stop=True)
                st_x = stpool.tile([128, SPAN], PVDT, tag="st")
                nc.scalar.activation(out=st_x, in_=st_ps,
                                     func=mybir.ActivationFunctionType.Exp)
                if kb == 0:
                    nc.vector.tensor_copy(out=acc, in_=f32view(st_x))
                else:
                    nc.vector.tensor_add(out=acc, in0=acc, in1=f32view(st_x))
                if prev_st is not None:
                    kbp, stp = prev_st
                    nc.tensor.matmul(ot_ps, mm(V_s[:, kbp, :]), mm(stp),
                                     start=(kbp == 0), stop=False)
                prev_st = (kb, st_x)
            kbp, stp = prev_st
            nc.tensor.matmul(ot_ps, mm(V_s[:, kbp, :]), mm(stp),
                             start=False, stop=True)

            outsp = outpool.tile([128, SPAN], F32, tag="out")
            if variant == "nonorm":
                nc.vector.tensor_copy(out=outsp, in_=ot_ps)
            else:
                # lb[d, q] = sum over partitions of acc, broadcast to all
                # 128 partitions, via one all-ones 128x128 matmul (exact f32)
                lb_ps = psB.tile([128, SPAN], F32, tag="small")
                nc.tensor.matmul(lb_ps, ones_mat, acc,
                                 start=True, stop=True)
                lb = lbpool.tile([128, SPAN], F32, tag="lb")
                nc.vector.reciprocal(out=lb, in_=lb_ps)
                nc.vector.tensor_mul(out=outsp, in0=ot_ps, in1=lb)
            nc.sync.dma_start(out=outT[:, ssl], in_=outsp)

    return nc


_CACHED = {}


def _get_nc(mm_dt=F32R):
    key = str(mm_dt)
    if key not in _CACHED:
        nc = build_nc(mm_dt)
        _split_excess_waits(nc)
        _CACHED[key] = nc
    return _CACHED[key]


def _round_tf32(a):
    """Round fp32 to tf32 (10-bit mantissa), round-to-nearest-even."""
    u = np.ascontiguousarray(a, np.float32).view(np.uint32)
    r = (u + np.uint32(0xFFF) + ((u >> np.uint32(13)) & np.uint32(1))) \
        & np.uint32(0xFFFFE000)
    return r.view(np.float32)


def _make_in_maps(x, Wq, bq, Wk, bk, Wv, bv, mm_dt=F32R):
    rnd = _round_tf32 if mm_dt == F32R else (
        lambda a: np.ascontiguousarray(a, np.float32))
    xT = rnd(np.transpose(np.asarray(x, np.float32), (0, 2, 1)))

    def warr(w):
        w = np.asarray(w, np.float32).reshape(EC, 128, D)
        return rnd(w.transpose(1, 0, 2).reshape(128, EC * D))

    Wq, Wk, Wv = warr(Wq), warr(Wk), warr(Wv)
    bqc = np.ascontiguousarray(np.asarray(bq, np.float32))
    bkc = np.ascontiguousarray(np.asarray(bk, np.float32))
    bv = np.ascontiguousarray(np.asarray(bv, np.float32))
    return [
        {"xT": np.ascontiguousarray(xT[b]), "Wq": Wq, "Wk": Wk, "Wv": Wv,
         "bqc": bqc, "bkc": bkc, "bv": bv}
        for b in range(B)
    ]


def kernel(x, Wq, bq, Wk, bk, Wv, bv, _trace=False, _mm_dt=None):
    from concourse.bass_utils import run_bass_kernel_spmd

    mm_dt = _mm_dt if _mm_dt is not None else F32R
    nc = _get_nc(mm_dt)
    in_maps = _make_in_maps(x, Wq, bq, Wk, bk, Wv, bv, mm_dt=mm_dt)
    res = run_bass_kernel_spmd(nc, in_maps, core_ids=list(range(B)),
                               trace=_trace)
    out = np.stack([np.ascontiguousarray(res.results[b]["outT"].T)
                    for b in range(B)])
    kernel._last_result = res
    return out
